# revision 2
# baseline (speedup 1.0000x reference)
"""Trainium2 Bass kernel for nn_Convolution_1176821039249.

Computes out = base_map * mean_k box_k(x) for k in {3,5,7,9,11,13,15} with
replicate padding, on 8 NeuronCores in a 4x2 spatial grid (1024x2048 per
core plus a 7-pixel halo on all sides, sliced host-side from the padded
full image, so no device-side halo exchange is needed).

Algorithm (per core):
  The total 2D kernel K(di,dj) = sum_k 1/(7k^2) 1[|di|<=k//2] 1[|dj|<=k//2]
  is decomposed over the horizontal "wing" basis
      T_0 = x(center),  T_m(j) = x(j-m) + x(j+m)   (m = 1..7)
  so that  out = sum_{b=0..7} P_b-vertical-band applied to T_b, where
      P_b(d) = sum_{k: k//2 >= max(b,|d|)} 1/(7k^2).
  Wings m=1..6 are ONE fused DVE tensor_tensor add (3D fan views with
  column-offset steps -1/+1); wing 7 is column-split between DVE and Pool
  to balance engine load.  The vertical pyramid bands are 8 PSUM-accumulated
  banded matmuls on the PE per 114-row tile; ACT drains PSUM to fp16, Pool
  multiplies by base_map, and the fp16 result is stored (host upconverts
  to fp32).
"""

import numpy as np

F16 = np.float16

H = W = 4096
PAD = 7
GR, GC = 4, 2               # core grid: 4 row-groups x 2 col-groups
RPC = H // GR               # 1024 output rows per core
CPC = W // GC               # 2048 output cols per core
SHARD_R = RPC + 2 * PAD     # 1038
SHARD_C = CPC + 2 * PAD     # 2062
TILE_M = 114                # output rows per tile (128 - 2*PAD)
N_TILES = 9                 # 8 * 114 + 112 = 1024
LAST_M = RPC - (N_TILES - 1) * TILE_M   # 112
CHUNK = 512                 # matmul N chunk (one PSUM bank of fp32)
N_CORES = 8
KERNEL_SIZES = (3, 5, 7, 9, 11, 13, 15)
DVE_W7 = 736                # columns of wing-7 computed on DVE; rest on Pool

_CACHE = {}


def _bands_np() -> np.ndarray:
    """lhsT band matrices, [128, 8*TILE_M] fp16.

    Band b column i row p holds P_b(p - i - 7): the vertical pyramid profile
    applied to wing tensor T_b.
    """
    w = {k: 1.0 / (7.0 * k * k) for k in KERNEL_SIZES}
    P = np.zeros((8, 15), dtype=np.float64)
    for b in range(8):
        for d in range(-7, 8):
            P[b, d + 7] = sum(w[k] for k in KERNEL_SIZES if k // 2 >= max(b, abs(d)))
    M = np.zeros((128, 8 * TILE_M), dtype=np.float64)
    for b in range(8):
        for i in range(TILE_M):
            for p in range(i, i + 15):
                M[p, b * TILE_M + i] = P[b, p - i]
    return M.astype(F16)


def _build_nc():
    import concourse.bass as bass
    import concourse.mybir as mybir
    import concourse.tile as tile

    dt = mybir.dt

    nc = bass.Bass()
    xb_d = nc.declare_dram_parameter("xb", [SHARD_R, SHARD_C], dt.float16, isOutput=False)
    base_d = nc.declare_dram_parameter("base", [RPC, CPC], dt.float16, isOutput=False)
    bands_d = nc.declare_dram_parameter("bands", [128, 8 * TILE_M], dt.float16, isOutput=False)
    out_d = nc.declare_dram_parameter("out", [RPC, CPC], dt.float16, isOutput=True)

    with tile.TileContext(nc) as tc:
        with (
            tc.tile_pool(name="const", bufs=1) as constp,
            tc.tile_pool(name="xin", bufs=3) as xpool,
            tc.tile_pool(name="wings", bufs=2) as wpool,
            tc.tile_pool(name="io", bufs=3) as iopool,
            tc.tile_pool(name="psum", bufs=2, space="PSUM") as psump,
        ):
            bands_sb = constp.tile([128, 8 * TILE_M], dt.float16, name="bands_sb")
            nc.sync.dma_start(bands_sb[:], bands_d[:])

            def fan(src, start, n, step, width):
                # [K, n, width] view: slice i covers columns
                # start + i*step .. +width (overlapping windows)
                v = src[:, start:start + width].unsqueeze(1)
                lst = v.ap
                lst[1] = (step, n)
                v.ap = lst
                return v

            pending = None  # (ps, bt, r0, M) awaiting drain+mul+store

            def epilogue():
                ps, bt, r0, M = pending
                acc = iopool.tile([128, CPC], dt.float16, tag="acc", name="acc")
                nc.scalar.copy(acc[:M, :], ps[:M, :])
                nc.gpsimd.tensor_mul(acc[:M, :], acc[:M, :], bt[:M, :])
                nc.sync.dma_start(out_d[r0:r0 + M, :], acc[:M, :])

            for t in range(N_TILES):
                M = TILE_M if t < N_TILES - 1 else LAST_M
                K = M + 2 * PAD
                r0 = t * TILE_M

                xt = xpool.tile([128, SHARD_C], dt.float16, tag="xt", name="xt")
                nc.sync.dma_start(xt[:K, :], xb_d[r0:r0 + K, :])
                bt = iopool.tile([128, CPC], dt.float16, tag="bt", name="bt")
                nc.sync.dma_start(bt[:M, :], base_d[r0:r0 + M, :])

                # wings m=1..6 fused: slice i reads cols (6-i ..) and (8+i ..),
                # i.e. x shifted by -(i+1) and +(i+1) around the PAD=7 center
                w6 = wpool.tile([128, 6, CPC], dt.float16, tag="w6", name="w6")
                nc.vector.tensor_add(
                    w6[:K], fan(xt[:K], 6, 6, -1, CPC), fan(xt[:K], 8, 6, 1, CPC))
                # wing 7 column-split between DVE and Pool for balance
                w7 = wpool.tile([128, CPC], dt.float16, tag="w7", name="w7")
                nc.vector.tensor_add(
                    w7[:K, :DVE_W7], xt[:K, 0:DVE_W7], xt[:K, 14:14 + DVE_W7])
                nc.gpsimd.tensor_add(
                    w7[:K, DVE_W7:], xt[:K, DVE_W7:CPC], xt[:K, 14 + DVE_W7:14 + CPC])

                # previous tile's PSUM drain / base-mul / store, emitted here so
                # Pool's in-order queue runs w7(t) before mul(t-1)
                if pending is not None:
                    epilogue()

                ps = psump.tile([128, CPC], dt.float32, tag="ps", name="ps")
                for b in range(8):
                    if b == 0:
                        rhs = xt[:K, PAD:PAD + CPC]
                    elif b < 7:
                        rhs = w6[:K, b - 1]
                    else:
                        rhs = w7[:K, :]
                    lhsT = bands_sb[:K, b * TILE_M:b * TILE_M + M]
                    for c in range(CPC // CHUNK):
                        nc.tensor.matmul(
                            ps[:M, c * CHUNK:(c + 1) * CHUNK],
                            lhsT,
                            rhs[:, c * CHUNK:(c + 1) * CHUNK],
                            start=(b == 0),
                            stop=(b == 7),
                        )
                pending = (ps, bt, r0, M)

            epilogue()
    return nc


def _split_sync_waits(nc):
    """Walrus codegen only supports one sync wait per instruction; hoist
    extra waits onto injected NoOps on the instruction's engine (identical
    semantics: the sequencer blocks at the NoOp first, then at the
    instruction).  DMA instructions are issued from their engine's
    sequencer stream, so the same hoisting applies to them.
    """
    import concourse.mybir as mybir

    n_nops = 0
    for fn in nc.m.functions:
        for bb in fn.blocks:
            new = []
            for inst in bb.instructions:
                si = inst.sync_info
                if si is not None and si.on_wait and len(si.on_wait) > 1:
                    waits = list(si.on_wait)
                    hoist, keep = waits[:-1], waits[-1:]
                    for w in hoist:
                        nop = mybir.InstNoOp(name=f"{inst.name}-w{n_nops}", ins=[], outs=[])
                        nop.engine = inst.engine
                        nop.sync_info = mybir.SyncInfo(on_wait=[w], on_update=[])
                        new.append(nop)
                        n_nops += 1
                    if hoist:
                        inst.sync_info = mybir.SyncInfo(
                            on_wait=keep, on_update=list(si.on_update))
                new.append(inst)
            bb.instructions = new
    return n_nops


def _get_nc():
    if "nc" not in _CACHE:
        nc = _build_nc()
        _split_sync_waits(nc)
        _CACHE["nc"] = nc
    return _CACHE["nc"]


def _run(x: np.ndarray, base_map: np.ndarray, trace: bool = False):
    from concourse.bass_utils import run_bass_kernel_spmd

    nc = _get_nc()
    xp = np.pad(np.asarray(x, dtype=np.float32), PAD, mode="edge").astype(F16)
    base16 = np.asarray(base_map, dtype=np.float32).astype(F16)
    bands = _bands_np()
    in_maps = []
    for c in range(N_CORES):
        gr, gc = divmod(c, GC)
        r0, c0 = gr * RPC, gc * CPC
        in_maps.append({
            "xb": np.ascontiguousarray(xp[r0:r0 + SHARD_R, c0:c0 + SHARD_C]),
            "base": np.ascontiguousarray(base16[r0:r0 + RPC, c0:c0 + CPC]),
            "bands": bands,
        })
    res = run_bass_kernel_spmd(nc, in_maps, list(range(N_CORES)), trace=trace)
    out = np.empty((H, W), dtype=np.float32)
    for c in range(N_CORES):
        gr, gc = divmod(c, GC)
        r0, c0 = gr * RPC, gc * CPC
        out[r0:r0 + RPC, c0:c0 + CPC] = res.results[c]["out"].astype(np.float32)
    return out[None, None], res


def kernel(x: np.ndarray, base_map: np.ndarray) -> np.ndarray:
    out, _ = _run(x, base_map, trace=False)
    return out
